# revision 7
# baseline (speedup 1.0000x reference)
"""Causal single-head attention on 8 trn2 cores, batch-data-parallel.

Computes, for each batch item b:
    Q = x[b] @ Wq + bq; K = x[b] @ Wk + bk; V = x[b] @ Wv + bv
    out[b] = softmax(causal_mask(Q K^T / sqrt(H))) @ V

Shapes: x [256, 256, 384], W* [384, 64], b* [64], out [256, 256, 64] fp32.
Sharding: batch axis split across 8 cores (32 items each), weights replicated.

All matmul operands are bfloat16 (PSUM accumulation stays fp32).

v2 layout (pairs processed in GROUPS of two, pairs A and B):
- K-projection is COL-TILED: Wk loads into array cols 0:63 for pair A
  (psum partitions 0:64) and cols 64:127 for pair B (partitions 64:128);
  the two M=64 matmuls run concurrently, halving the K-pass time that was
  previously a half-empty full pass per pair.
- Pair B's Q relocates to partitions 64:128 during the psum->sbuf
  conversion (DVE copies support partition-base shifts); K_B lands there
  naturally via the col-tiling. Scores for pair B then run as ROW-TILE
  (64,0) concurrently with pair A's (0,0): QK^T for both pairs in the
  time of one. (Row-tiled matmuls must write DIFFERENT psum banks, and
  transpose-mode instructions must all share one input base partition —
  mixing bases crashes the NEFF at runtime; hence V of both pairs stays
  at partitions 64:128 and only Q_B moves.)
- Out-product is TRANSPOSED: o^T[h, t] with lhsT = V' = [V|1|1] (66 cols
  -> cheap LDWEIGHTS) and rhs = wei^T blocks; 2 matmuls per item
  (N=256 for (s0,{t0,t1}), N=128 accumulating (s1,t1)). The ones columns
  make rows 64:66 the softmax denominator; the host does the divide.
- Output per pair is o^T [66, 512] fp32->bf16, one DMA of 66 descriptors.

Inherited from the v1 baseline (see git history / kernel_baseline.py):
x^T pair-major host layout (one 128-descriptor DMA per pair), packed
weight blob, exp-then-mask causal handling on GpSimd, ones-column
denominator trick, 1-group software-pipeline skew, first-chunk-first
DMA ordering.
"""

import ml_dtypes
import numpy as np

import concourse.bacc as bacc
import concourse.mybir as mybir
import concourse.tile as tile
from concourse import bass_utils
from concourse.masks import make_identity

N_CORES = 8
B_FULL, T, C, H = 256, 256, 384, 64
B_SHARD = B_FULL // N_CORES  # 32
NPAIR = B_SHARD // 2  # 16
NGROUP = NPAIR // 2  # 8
F32 = mybir.dt.float32
BF16 = mybir.dt.bfloat16
SCALE = float(H) ** -0.5

EXP = mybir.ActivationFunctionType.Exp


def _build(has_bias):
    MMT = BF16
    nc = bacc.Bacc("TRN2", target_bir_lowering=False, debug=False, num_devices=N_CORES)

    # x^T pair-major: row (bp*128+p), cols (k*512+t) -> x[pair bp][t][k*128+p]
    xT_d = nc.dram_tensor("xT", [NPAIR * 128, 3 * 512], MMT, kind="ExternalInput").ap()
    # weight blob, cols: [c*128+0:64) = Wq chunk c, [c*128+64:128) = Wv
    # chunk c; [384+c*64 : 384+(c+1)*64) = Wk chunk c (col-tiled for A+B)
    wp_d = nc.dram_tensor("wpack", [128, 576], MMT, kind="ExternalInput").ap()
    # biases: col0 = [bq;bv], col1 = [bk;bk]
    bias_d = (
        nc.dram_tensor("bias", [128, 2], F32, kind="ExternalInput").ap()
        if has_bias
        else None
    )
    # out^T pair-major: row (bp*66+h), col (bi*256+t); rows 64:66 of each
    # 66-chunk are the softmax denominator (host divides).
    out_d = nc.dram_tensor("out", [NPAIR * 66, 512], MMT, kind="ExternalOutput").ap()

    xT_r = xT_d.rearrange("(b p) (k t) -> b p k t", p=128, k=3)
    out_r = out_d.rearrange("(b h) t -> b h t", h=66)

    with tile.TileContext(nc) as tc:
        with (
            tc.tile_pool(name="singles", bufs=1) as singles,
            tc.tile_pool(name="sb", bufs=2) as sb,
            tc.tile_pool(name="sbx", bufs=2) as sbx,
            tc.tile_pool(name="ps_qv", bufs=1, space="PSUM") as ps_qv,
            tc.tile_pool(name="ps_k", bufs=1, space="PSUM") as ps_k,
            tc.tile_pool(name="ps_s", bufs=1, space="PSUM") as ps_s,
            tc.tile_pool(name="ps_v", bufs=1, space="PSUM") as ps_v,
            tc.tile_pool(name="ps_o", bufs=1, space="PSUM") as ps_o,
        ):
            # ---- one-time setup ----
            # pair A0 chunk 0 first, then the first weight chunk: these gate
            # the first matmul, so their descriptors hit the queues first.
            xtA0 = sbx.tile([128, 3, 512], MMT, tag="xtA")
            nc.sync.dma_start(xtA0[:, 0, :], xT_r[0, :, 0, :])
            wt = singles.tile([128, 576], MMT)
            nc.sync.dma_start(wt[:, 0:128], wp_d[:, 0:128])
            nc.sync.dma_start(wt[:, 128:576], wp_d[:, 128:576])
            wqv = wt[:, 0:384].rearrange("p (c m) -> p c m", c=3)  # [128,3,128]
            wkk = wt[:, 384:576].rearrange("p (c m) -> p c m", c=3)  # [128,3,64]
            if has_bias:
                bias_t = singles.tile([128, 2], F32)
                nc.sync.dma_start(bias_t[:], bias_d[:])

            identf = singles.tile([128, 128], F32)
            make_identity(nc, identf[:])
            ident = singles.tile([128, 128], MMT)
            nc.vector.tensor_copy(ident[:], identf[:])

            def stage_a(g):
                """x DMAs + projections + psum->sbuf conversions for group g
                (pairs 2g and 2g+1)."""
                if g == 0:
                    xtA = xtA0
                    nc.sync.dma_start(xtA[:, 1:3, :], xT_r[0, :, 1:3, :])
                else:
                    xtA = sbx.tile([128, 3, 512], MMT, tag="xtA")
                    nc.sync.dma_start(xtA[:], xT_r[2 * g])
                xtB = sbx.tile([128, 3, 512], MMT, tag="xtB")
                nc.sync.dma_start(xtB[:], xT_r[2 * g + 1])

                qv_ps = ps_qv.tile([128, 2, 512], F32, tag="qv_ps")
                k_ps = ps_k.tile([128, 512], F32, tag="k_ps")
                for c in range(3):
                    nc.tensor.matmul(
                        qv_ps[:, 0, :], wqv[:, c, :], xtA[:, c, :],
                        start=(c == 0), stop=(c == 2),
                    )
                for c in range(3):
                    nc.tensor.matmul(
                        qv_ps[:, 1, :], wqv[:, c, :], xtB[:, c, :],
                        start=(c == 0), stop=(c == 2),
                    )
                # col-tiled K: pair A -> psum parts 0:64 (array cols 0:63),
                # pair B -> parts 64:128 (cols 64:127); concurrent M=64 MMs.
                # skip_group_check: the sim's zero-region tracker is blind to
                # the disjoint partition bases of the two interleaved groups;
                # hardware has_written bits are per-element.
                for c in range(3):
                    nc.tensor.matmul(
                        k_ps[0:64, :], wkk[:, c, :], xtA[:, c, :],
                        start=(c == 0), stop=(c == 2), skip_group_check=True,
                    )
                    nc.tensor.matmul(
                        k_ps[64:128, :], wkk[:, c, :], xtB[:, c, :],
                        start=(c == 0), stop=(c == 2), skip_group_check=True,
                    )

                # pair A: straight copy; Q_A parts 0:64, V_A parts 64:128.
                # pair B: V_B (psum parts 64:128) stays at 64:128 so ALL
                # V-transposes share base 64 (mixed transpose bases crash);
                # Q_B relocates psum parts 0:64 -> sbuf parts 64:128 so the
                # B scores can row-tile on array rows 64:127.
                qv2A = sb.tile([128, 512], MMT, tag="qv2A")
                qv2B = sb.tile([128, 512], MMT, tag="qv2B")  # V_B at parts 64:128
                qB = sb.tile([128, 512], MMT, tag="qB")  # Q_B at parts 64:128
                k2 = sb.tile([128, 512], MMT, tag="k2")
                if has_bias:
                    nc.vector.tensor_single_scalar(
                        qv2A[:], qv_ps[:, 0, :], bias_t[:, 0:1], op=mybir.AluOpType.add
                    )
                    nc.vector.tensor_single_scalar(
                        qB[64:128, :], qv_ps[0:64, 1, :], bias_t[0:64, 0:1],
                        op=mybir.AluOpType.add,
                    )
                    nc.vector.tensor_single_scalar(
                        qv2B[64:128, :], qv_ps[64:128, 1, :], bias_t[64:128, 0:1],
                        op=mybir.AluOpType.add,
                    )
                    nc.scalar.tensor_single_scalar(
                        k2[:], k_ps[:], bias_t[:, 1:2], op=mybir.AluOpType.add
                    )
                else:
                    # zero biases: plain converting copies, split per t-half
                    # so item0's consumers unblock half a copy earlier.
                    nc.vector.tensor_copy(qv2A[:, 0:256], qv_ps[:, 0, 0:256])
                    nc.vector.tensor_copy(qv2A[:, 256:512], qv_ps[:, 0, 256:512])
                    nc.vector.tensor_copy(qB[64:128, 0:256], qv_ps[0:64, 1, 0:256])
                    nc.vector.tensor_copy(qB[64:128, 256:512], qv_ps[0:64, 1, 256:512])
                    nc.vector.tensor_copy(qv2B[64:128, 0:256], qv_ps[64:128, 1, 0:256])
                    nc.vector.tensor_copy(qv2B[64:128, 256:512], qv_ps[64:128, 1, 256:512])
                    nc.scalar.copy(k2[:, 0:256], k_ps[:, 0:256])
                    nc.scalar.copy(k2[:, 256:512], k_ps[:, 256:512])
                return qv2A, qv2B, qB, k2

            def stage_b(g, qv2A, qv2B, qB, k2):
                """scores/softmax/V for both pairs of group g."""
                v_ps = ps_v.tile([128, 2, 4, 64], MMT, tag="v_ps")
                W_A = sb.tile([128, 2, 3, 128], MMT, tag="WA")
                W_B = sb.tile([128, 2, 3, 128], MMT, tag="WB")

                def transp(pa, q):
                    # V of both pairs sits at parts 64:128 (uniform base:
                    # transpose-mode instructions must not mix bases)
                    qv2 = qv2A if pa == 0 else qv2B
                    nc.tensor.transpose(
                        v_ps[:, pa, q, :],
                        qv2[64:128, q * 128 : (q + 1) * 128],
                        ident[64:128, 64:128],
                    )

                def scores(pa, bi):
                    # row-tile: pair A on array rows 0:63, pair B on 64:127
                    base = 0 if pa == 0 else 64
                    qsrc = qv2A if pa == 0 else qB
                    toff = bi * 256
                    s_ps = ps_s.tile([128, 3, 128], F32, tag=f"s{pa}")
                    kT = k2[base : base + 64, toff : toff + 256]
                    qT = qsrc[base : base + 64, toff : toff + 256]
                    nc.tensor.matmul(
                        s_ps[:, 0:2, :], kT[:, 0:128], qT, start=True, stop=True
                    )
                    nc.tensor.matmul(
                        s_ps[:, 2, :], kT[:, 128:256], qT[:, 128:256],
                        start=True, stop=True,
                    )
                    return s_ps

                def softmax(pa, bi, s_ps):
                    W = W_A if pa == 0 else W_B
                    nc.scalar.activation(W[:, bi, :, :], s_ps[:], EXP, scale=SCALE)
                    nc.gpsimd.affine_select(
                        out=W[:, bi, 0:3:2, :],
                        in_=W[:, bi, 0:3:2, :],
                        compare_op=mybir.AluOpType.is_ge,
                        fill=0.0,
                        base=0,
                        pattern=[[0, 2], [1, 128]],  # keep where (-s + t) >= 0
                        channel_multiplier=-1,
                    )

                sA0 = scores(0, 0)
                sB0 = scores(1, 0)
                transp(0, 0)
                transp(0, 1)
                softmax(0, 0, sA0)
                transp(1, 0)
                transp(1, 1)
                softmax(1, 0, sB0)
                sA1 = scores(0, 1)
                sB1 = scores(1, 1)
                transp(0, 2)
                transp(0, 3)
                softmax(0, 1, sA1)
                transp(1, 2)
                transp(1, 3)
                softmax(1, 1, sB1)

                v_sb = sb.tile([128, 2, 2, 2, 66], MMT, tag="v_sb", bufs=3)
                nc.vector.tensor_copy(
                    v_sb[:, :, :, :, 0:64].rearrange("p a i n h -> p (a i n) h"),
                    v_ps[:].rearrange("p a q h -> p (a q) h"),
                )
                # ones columns -> out^T rows 64:66 = softmax denominator
                # (free size is only 8 elements; memset every group)
                nc.vector.memset(v_sb[:, :, :, :, 64:66], 1.0)
                return W_A, W_B, v_sb

            def stage_c(g, W_A, W_B, v_sb):
                """transposed out-product + store for both pairs — emitted
                AFTER the next group's projections so those fill the PE
                while exp/mask run."""
                for pa, W in ((0, W_A), (1, W_B)):
                    o_ps = ps_o.tile([66, 512], F32, tag=f"o{pa}")
                    for bi in range(2):
                        toff = bi * 256
                        # o^T[h, t0:t1] = V'(s0)^T @ [wei^T(s0,t0)|wei^T(s0,t1)]
                        nc.tensor.matmul(
                            o_ps[:, toff : toff + 256],
                            v_sb[:, pa, bi, 0, :],
                            W[:, bi, 0:2, :],
                            start=True, stop=False,
                        )
                        # o^T[h, t1] += V'(s1)^T @ wei^T(s1,t1)
                        nc.tensor.matmul(
                            o_ps[:, toff + 128 : toff + 256],
                            v_sb[:, pa, bi, 1, :],
                            W[:, bi, 2, :],
                            start=False, stop=True,
                        )

                    o_sb = sb.tile([66, 512], MMT, tag=f"os{pa}")
                    bp = 2 * g + pa
                    if g == NGROUP - 1:
                        # last group: store per item so item0 drains while
                        # item1 finishes
                        for bi in range(2):
                            sl = slice(bi * 256, bi * 256 + 256)
                            nc.vector.tensor_copy(o_sb[:, sl], o_ps[:, sl])
                            nc.sync.dma_start(out_r[bp, :, sl], o_sb[:, sl])
                    else:
                        nc.vector.tensor_copy(o_sb[:], o_ps[:])
                        nc.sync.dma_start(out_r[bp], o_sb[:])

            # software pipeline with a 1-group skew
            pend = stage_a(0)
            for g in range(NGROUP):
                nxt = stage_a(g + 1) if g + 1 < NGROUP else None
                wv = stage_b(g, *pend)
                stage_c(g, *wv)
                pend = nxt

    nc.compile()
    return nc


_CACHE = {}


def get_nc(has_bias=False):
    if has_bias not in _CACHE:
        _CACHE[has_bias] = _build(has_bias)
    return _CACHE[has_bias]


def make_in_maps(x, Wq, bq, Wk, bk, Wv, bv):
    bf16 = ml_dtypes.bfloat16
    x = np.asarray(x, dtype=np.float32)
    Wq = np.asarray(Wq, dtype=np.float32)
    Wk = np.asarray(Wk, dtype=np.float32)
    Wv = np.asarray(Wv, dtype=np.float32)
    wp = np.zeros((128, 576), dtype=np.float32)
    for c in range(3):
        wc = slice(c * 128, (c + 1) * 128)
        wp[:, c * 128 : c * 128 + 64] = Wq[wc]
        wp[:, c * 128 + 64 : c * 128 + 128] = Wv[wc]
        wp[:, 384 + c * 64 : 384 + (c + 1) * 64] = Wk[wc]
    wp = np.ascontiguousarray(wp.astype(bf16))
    bq = np.asarray(bq, dtype=np.float32).ravel()
    bk = np.asarray(bk, dtype=np.float32).ravel()
    bv = np.asarray(bv, dtype=np.float32).ravel()
    has_bias = bool(np.any(bq) or np.any(bk) or np.any(bv))
    bias = np.zeros((128, 2), dtype=np.float32)
    bias[0:64, 0] = bq
    bias[64:128, 0] = bv
    bias[0:64, 1] = bk
    bias[64:128, 1] = bk

    in_maps = []
    for i in range(N_CORES):
        shard = x[i * B_SHARD : (i + 1) * B_SHARD]  # [32, 256, 384]
        pairs = shard.reshape(NPAIR, 512, C)  # t within pair = bi*256 + t'
        xTc = pairs.transpose(0, 2, 1)  # [b, C, t]
        xT = xTc.reshape(NPAIR, 3, 128, 512).transpose(0, 2, 1, 3)
        xT = np.ascontiguousarray(xT.astype(bf16)).reshape(NPAIR * 128, 3 * 512)
        m = {"xT": xT, "wpack": wp}
        if has_bias:
            m["bias"] = bias
        in_maps.append(m)
    return in_maps


def kernel(x, Wq, bq, Wk, bk, Wv, bv):
    in_maps = make_in_maps(x, Wq, bq, Wk, bk, Wv, bv)
    nc = get_nc("bias" in in_maps[0])
    res = bass_utils.run_bass_kernel_spmd(nc, in_maps, core_ids=list(range(N_CORES)))
    outs = []
    for i in range(N_CORES):
        r = res.results[i]["out"].reshape(NPAIR, 66, 2, 256).astype(np.float32)
        r = r[:, 0:64] / r[:, 64:65]  # softmax denominator divide
        # [bp, h, bi, t] -> item 2bp+bi at [t, h]
        outs.append(
            np.ascontiguousarray(r.transpose(0, 2, 3, 1)).reshape(B_SHARD, T, H)
        )
    return np.concatenate(outs, axis=0)


# revision 8
# speedup vs baseline: 1.0805x; 1.0805x over previous
"""Causal single-head attention on 8 trn2 cores, batch-data-parallel.

Computes, for each batch item b:
    Q = x[b] @ Wq + bq; K = x[b] @ Wk + bk; V = x[b] @ Wv + bv
    out[b] = softmax(causal_mask(Q K^T / sqrt(H))) @ V

Shapes: x [256, 256, 384], W* [384, 64], b* [64], out [256, 256, 64] fp32.
Sharding: batch axis split across 8 cores (32 items each), weights replicated.

All matmul operands are bfloat16 (PSUM accumulation stays fp32).

v2 layout (pairs processed in GROUPS of two, pairs A and B):
- K-projection is COL-TILED: Wk loads into array cols 0:63 for pair A
  (psum partitions 0:64) and cols 64:127 for pair B (partitions 64:128);
  the two M=64 matmuls run concurrently, halving the K-pass time that was
  previously a half-empty full pass per pair.
- Pair B's Q relocates to partitions 64:128 during the psum->sbuf
  conversion (DVE copies support partition-base shifts); K_B lands there
  naturally via the col-tiling. Scores for pair B then run as ROW-TILE
  (64,0) concurrently with pair A's (0,0): QK^T for both pairs in the
  time of one. (Row-tiled matmuls must write DIFFERENT psum banks, and
  transpose-mode instructions must all share one input base partition —
  mixing bases crashes the NEFF at runtime; hence V of both pairs stays
  at partitions 64:128 and only Q_B moves.)
- Out-product is TRANSPOSED: o^T[h, t] with lhsT = V' = [V|1|1] (66 cols
  -> cheap LDWEIGHTS) and rhs = wei^T blocks; 2 matmuls per item
  (N=256 for (s0,{t0,t1}), N=128 accumulating (s1,t1)). The ones columns
  make rows 64:66 the softmax denominator; the host does the divide.
- Output per pair is o^T [66, 512] fp32->bf16, one DMA of 66 descriptors.

Inherited from the v1 baseline (see git history / kernel_baseline.py):
x^T pair-major host layout (one 128-descriptor DMA per pair), packed
weight blob, exp-then-mask causal handling on GpSimd, ones-column
denominator trick, 1-group software-pipeline skew, first-chunk-first
DMA ordering.
"""

import ml_dtypes
import numpy as np

import concourse.bacc as bacc
import concourse.mybir as mybir
import concourse.tile as tile
from concourse import bass_utils
from concourse.masks import make_identity

N_CORES = 8
B_FULL, T, C, H = 256, 256, 384, 64
B_SHARD = B_FULL // N_CORES  # 32
NPAIR = B_SHARD // 2  # 16
NGROUP = NPAIR // 2  # 8
F32 = mybir.dt.float32
BF16 = mybir.dt.bfloat16
SCALE = float(H) ** -0.5

EXP = mybir.ActivationFunctionType.Exp


def _build(has_bias):
    MMT = BF16
    nc = bacc.Bacc("TRN2", target_bir_lowering=False, debug=False, num_devices=N_CORES)

    # x^T pair-major: row (bp*128+p), cols (k*512+t) -> x[pair bp][t][k*128+p]
    xT_d = nc.dram_tensor("xT", [NPAIR * 128, 3 * 512], MMT, kind="ExternalInput").ap()
    # weight blob, cols: [c*128+0:64) = Wq chunk c, [c*128+64:128) = Wv
    # chunk c; [384+c*64 : 384+(c+1)*64) = Wk chunk c (col-tiled for A+B)
    wp_d = nc.dram_tensor("wpack", [128, 576], MMT, kind="ExternalInput").ap()
    # biases: col0 = [bq;bv], col1 = [bk;bk]
    bias_d = (
        nc.dram_tensor("bias", [128, 2], F32, kind="ExternalInput").ap()
        if has_bias
        else None
    )
    # out^T pair-major: row (bp*66+h), col (bi*256+t); rows 64:66 of each
    # 66-chunk are the softmax denominator (host divides).
    out_d = nc.dram_tensor("out", [NPAIR * 66, 512], MMT, kind="ExternalOutput").ap()

    xT_r = xT_d.rearrange("(b p) (k t) -> b p k t", p=128, k=3)
    out_r = out_d.rearrange("(b h) t -> b h t", h=66)

    with tile.TileContext(nc) as tc:
        with (
            tc.tile_pool(name="singles", bufs=1) as singles,
            tc.tile_pool(name="sb", bufs=2) as sb,
            tc.tile_pool(name="sbx", bufs=2) as sbx,
            tc.tile_pool(name="ps_qv", bufs=1, space="PSUM") as ps_qv,
            tc.tile_pool(name="ps_k", bufs=1, space="PSUM") as ps_k,
            tc.tile_pool(name="ps_s", bufs=1, space="PSUM") as ps_s,
            tc.tile_pool(name="ps_v", bufs=1, space="PSUM") as ps_v,
            tc.tile_pool(name="ps_o", bufs=1, space="PSUM") as ps_o,
        ):
            # ---- one-time setup ----
            # pair A0 chunk 0 first, then the first weight chunk: these gate
            # the first matmul, so their descriptors hit the queues first.
            xtA0 = sbx.tile([128, 3, 512], MMT, tag="xtA")
            nc.sync.dma_start(xtA0[:, 0, :], xT_r[0, :, 0, :])
            wt = singles.tile([128, 576], MMT)
            nc.sync.dma_start(wt[:, 0:128], wp_d[:, 0:128])
            nc.sync.dma_start(wt[:, 128:576], wp_d[:, 128:576])
            wqv = wt[:, 0:384].rearrange("p (c m) -> p c m", c=3)  # [128,3,128]
            wkk = wt[:, 384:576].rearrange("p (c m) -> p c m", c=3)  # [128,3,64]
            if has_bias:
                bias_t = singles.tile([128, 2], F32)
                nc.sync.dma_start(bias_t[:], bias_d[:])

            identf = singles.tile([128, 128], F32)
            make_identity(nc, identf[:])
            ident = singles.tile([128, 128], MMT)
            nc.vector.tensor_copy(ident[:], identf[:])

            def stage_a(g):
                """x DMAs + projections + psum->sbuf conversions for group g
                (pairs 2g and 2g+1)."""
                if g == 0:
                    xtA = xtA0
                    nc.sync.dma_start(xtA[:, 1:3, :], xT_r[0, :, 1:3, :])
                else:
                    xtA = sbx.tile([128, 3, 512], MMT, tag="xtA")
                    nc.sync.dma_start(xtA[:], xT_r[2 * g])
                xtB = sbx.tile([128, 3, 512], MMT, tag="xtB")
                nc.sync.dma_start(xtB[:], xT_r[2 * g + 1])

                qv_ps = ps_qv.tile([128, 2, 512], F32, tag="qv_ps")
                k_ps = ps_k.tile([128, 512], F32, tag="k_ps")
                for c in range(3):
                    nc.tensor.matmul(
                        qv_ps[:, 0, :], wqv[:, c, :], xtA[:, c, :],
                        start=(c == 0), stop=(c == 2),
                    )
                for c in range(3):
                    nc.tensor.matmul(
                        qv_ps[:, 1, :], wqv[:, c, :], xtB[:, c, :],
                        start=(c == 0), stop=(c == 2),
                    )
                # col-tiled K: pair A -> psum parts 0:64 (array cols 0:63),
                # pair B -> parts 64:128 (cols 64:127); concurrent M=64 MMs.
                # skip_group_check: the sim's zero-region tracker is blind to
                # the disjoint partition bases of the two interleaved groups;
                # hardware has_written bits are per-element.
                for c in range(3):
                    nc.tensor.matmul(
                        k_ps[0:64, :], wkk[:, c, :], xtA[:, c, :],
                        start=(c == 0), stop=(c == 2), skip_group_check=True,
                    )
                    nc.tensor.matmul(
                        k_ps[64:128, :], wkk[:, c, :], xtB[:, c, :],
                        start=(c == 0), stop=(c == 2), skip_group_check=True,
                    )

                # Conversions as single 512-free ops: DVE/ACT per-op fixed
                # cost is ~340ns, so op count dominates, not lanes.
                # pair A: straight copy; Q_A parts 0:64, V_A parts 64:128.
                # pair B: straight copy too (V_B lands at 64:128 so ALL
                # V-transposes share base 64 — mixed transpose bases crash
                # the NEFF); Q_B is then relocated parts 0:64 -> 64:128 by
                # GPSIMD (sbuf->sbuf, the one engine with idle capacity) so
                # the B scores can row-tile on array rows 64:127.
                qv2A = sb.tile([128, 512], MMT, tag="qv2A")
                qv2B = sb.tile([128, 512], MMT, tag="qv2B")
                qB = sb.tile([128, 512], MMT, tag="qB")  # Q_B at parts 64:128
                k2 = sb.tile([128, 512], MMT, tag="k2")
                if has_bias:
                    nc.vector.tensor_single_scalar(
                        qv2A[:], qv_ps[:, 0, :], bias_t[:, 0:1], op=mybir.AluOpType.add
                    )
                    nc.vector.tensor_single_scalar(
                        qv2B[:], qv_ps[:, 1, :], bias_t[:, 0:1], op=mybir.AluOpType.add
                    )
                    nc.scalar.tensor_single_scalar(
                        k2[:], k_ps[:], bias_t[:, 1:2], op=mybir.AluOpType.add
                    )
                else:
                    nc.vector.tensor_copy(qv2A[:], qv_ps[:, 0, :])
                    nc.vector.tensor_copy(qv2B[:], qv_ps[:, 1, :])
                    nc.scalar.copy(k2[:], k_ps[:])
                nc.gpsimd.tensor_copy(qB[64:128, :], qv2B[0:64, :])
                return qv2A, qv2B, qB, k2

            def stage_b(g, qv2A, qv2B, qB, k2):
                """scores/softmax/V for both pairs of group g."""
                v_ps = ps_v.tile([128, 2, 4, 64], MMT, tag="v_ps")
                W_A = sb.tile([128, 2, 3, 128], MMT, tag="WA")
                W_B = sb.tile([128, 2, 3, 128], MMT, tag="WB")

                def transp(pa, q):
                    # V of both pairs sits at parts 64:128 (uniform base:
                    # transpose-mode instructions must not mix bases)
                    qv2 = qv2A if pa == 0 else qv2B
                    nc.tensor.transpose(
                        v_ps[:, pa, q, :],
                        qv2[64:128, q * 128 : (q + 1) * 128],
                        ident[64:128, 64:128],
                    )

                def scores(pa, bi):
                    # row-tile: pair A on array rows 0:63, pair B on 64:127
                    base = 0 if pa == 0 else 64
                    qsrc = qv2A if pa == 0 else qB
                    toff = bi * 256
                    s_ps = ps_s.tile([128, 3, 128], F32, tag=f"s{pa}")
                    kT = k2[base : base + 64, toff : toff + 256]
                    qT = qsrc[base : base + 64, toff : toff + 256]
                    nc.tensor.matmul(
                        s_ps[:, 0:2, :], kT[:, 0:128], qT, start=True, stop=True
                    )
                    nc.tensor.matmul(
                        s_ps[:, 2, :], kT[:, 128:256], qT[:, 128:256],
                        start=True, stop=True,
                    )
                    return s_ps

                def softmax(pa, bi, s_ps):
                    W = W_A if pa == 0 else W_B
                    nc.scalar.activation(W[:, bi, :, :], s_ps[:], EXP, scale=SCALE)
                    nc.gpsimd.affine_select(
                        out=W[:, bi, 0:3:2, :],
                        in_=W[:, bi, 0:3:2, :],
                        compare_op=mybir.AluOpType.is_ge,
                        fill=0.0,
                        base=0,
                        pattern=[[0, 2], [1, 128]],  # keep where (-s + t) >= 0
                        channel_multiplier=-1,
                    )

                sA0 = scores(0, 0)
                sB0 = scores(1, 0)
                transp(0, 0)
                transp(0, 1)
                softmax(0, 0, sA0)
                transp(1, 0)
                transp(1, 1)
                softmax(1, 0, sB0)
                sA1 = scores(0, 1)
                sB1 = scores(1, 1)
                transp(0, 2)
                transp(0, 3)
                softmax(0, 1, sA1)
                transp(1, 2)
                transp(1, 3)
                softmax(1, 1, sB1)

                v_sb = sb.tile([128, 2, 2, 2, 66], MMT, tag="v_sb", bufs=3)
                nc.vector.tensor_copy(
                    v_sb[:, :, :, :, 0:64].rearrange("p a i n h -> p (a i n) h"),
                    v_ps[:].rearrange("p a q h -> p (a q) h"),
                )
                # ones columns -> out^T rows 64:66 = softmax denominator
                # (free size is only 8 elements; memset every group)
                nc.vector.memset(v_sb[:, :, :, :, 64:66], 1.0)
                return W_A, W_B, v_sb

            def stage_c(g, W_A, W_B, v_sb):
                """transposed out-product + store for both pairs — emitted
                AFTER the next group's projections so those fill the PE
                while exp/mask run."""
                for pa, W in ((0, W_A), (1, W_B)):
                    o_ps = ps_o.tile([66, 512], F32, tag=f"o{pa}")
                    for bi in range(2):
                        toff = bi * 256
                        # o^T[h, t0:t1] = V'(s0)^T @ [wei^T(s0,t0)|wei^T(s0,t1)]
                        nc.tensor.matmul(
                            o_ps[:, toff : toff + 256],
                            v_sb[:, pa, bi, 0, :],
                            W[:, bi, 0:2, :],
                            start=True, stop=False,
                        )
                        # o^T[h, t1] += V'(s1)^T @ wei^T(s1,t1)
                        nc.tensor.matmul(
                            o_ps[:, toff + 128 : toff + 256],
                            v_sb[:, pa, bi, 1, :],
                            W[:, bi, 2, :],
                            start=False, stop=True,
                        )

                    o_sb = sb.tile([66, 512], MMT, tag=f"os{pa}")
                    bp = 2 * g + pa
                    cp = nc.vector.tensor_copy if pa == 0 else nc.scalar.copy
                    if g == NGROUP - 1:
                        # last group: store per item so item0 drains while
                        # item1 finishes
                        for bi in range(2):
                            sl = slice(bi * 256, bi * 256 + 256)
                            cp(o_sb[:, sl], o_ps[:, sl])
                            nc.sync.dma_start(out_r[bp, :, sl], o_sb[:, sl])
                    else:
                        cp(o_sb[:], o_ps[:])
                        nc.sync.dma_start(out_r[bp], o_sb[:])

            # software pipeline with a 1-group skew
            pend = stage_a(0)
            for g in range(NGROUP):
                nxt = stage_a(g + 1) if g + 1 < NGROUP else None
                wv = stage_b(g, *pend)
                stage_c(g, *wv)
                pend = nxt

    nc.compile()
    return nc


_CACHE = {}


def get_nc(has_bias=False):
    if has_bias not in _CACHE:
        _CACHE[has_bias] = _build(has_bias)
    return _CACHE[has_bias]


def make_in_maps(x, Wq, bq, Wk, bk, Wv, bv):
    bf16 = ml_dtypes.bfloat16
    x = np.asarray(x, dtype=np.float32)
    Wq = np.asarray(Wq, dtype=np.float32)
    Wk = np.asarray(Wk, dtype=np.float32)
    Wv = np.asarray(Wv, dtype=np.float32)
    wp = np.zeros((128, 576), dtype=np.float32)
    for c in range(3):
        wc = slice(c * 128, (c + 1) * 128)
        wp[:, c * 128 : c * 128 + 64] = Wq[wc]
        wp[:, c * 128 + 64 : c * 128 + 128] = Wv[wc]
        wp[:, 384 + c * 64 : 384 + (c + 1) * 64] = Wk[wc]
    wp = np.ascontiguousarray(wp.astype(bf16))
    bq = np.asarray(bq, dtype=np.float32).ravel()
    bk = np.asarray(bk, dtype=np.float32).ravel()
    bv = np.asarray(bv, dtype=np.float32).ravel()
    has_bias = bool(np.any(bq) or np.any(bk) or np.any(bv))
    bias = np.zeros((128, 2), dtype=np.float32)
    bias[0:64, 0] = bq
    bias[64:128, 0] = bv
    bias[0:64, 1] = bk
    bias[64:128, 1] = bk

    in_maps = []
    for i in range(N_CORES):
        shard = x[i * B_SHARD : (i + 1) * B_SHARD]  # [32, 256, 384]
        pairs = shard.reshape(NPAIR, 512, C)  # t within pair = bi*256 + t'
        xTc = pairs.transpose(0, 2, 1)  # [b, C, t]
        xT = xTc.reshape(NPAIR, 3, 128, 512).transpose(0, 2, 1, 3)
        xT = np.ascontiguousarray(xT.astype(bf16)).reshape(NPAIR * 128, 3 * 512)
        m = {"xT": xT, "wpack": wp}
        if has_bias:
            m["bias"] = bias
        in_maps.append(m)
    return in_maps


def kernel(x, Wq, bq, Wk, bk, Wv, bv):
    in_maps = make_in_maps(x, Wq, bq, Wk, bk, Wv, bv)
    nc = get_nc("bias" in in_maps[0])
    res = bass_utils.run_bass_kernel_spmd(nc, in_maps, core_ids=list(range(N_CORES)))
    outs = []
    for i in range(N_CORES):
        r = res.results[i]["out"].reshape(NPAIR, 66, 2, 256).astype(np.float32)
        r = r[:, 0:64] / r[:, 64:65]  # softmax denominator divide
        # [bp, h, bi, t] -> item 2bp+bi at [t, h]
        outs.append(
            np.ascontiguousarray(r.transpose(0, 2, 3, 1)).reshape(B_SHARD, T, H)
        )
    return np.concatenate(outs, axis=0)


# revision 9
# speedup vs baseline: 1.2840x; 1.1883x over previous
"""Causal single-head attention on 8 trn2 cores, batch-data-parallel.

Computes, for each batch item b:
    Q = x[b] @ Wq + bq; K = x[b] @ Wk + bk; V = x[b] @ Wv + bv
    out[b] = softmax(causal_mask(Q K^T / sqrt(H))) @ V

Shapes: x [256, 256, 384], W* [384, 64], b* [64], out [256, 256, 64] fp32.
Sharding: batch axis split across 8 cores (32 items each), weights replicated.

All matmul operands are bfloat16 (PSUM accumulation stays fp32); rel err
~5e-3 against the fp32 reference (gate 2e-2).

v4 structure = v1 baseline + chunk-major projections over 2-pair groups:
- Pairs are processed in GROUPS of two (A, B). The projection matmuls run
  CHUNK-MAJOR: qvA-c0, qvB-c0, qvA-c1, qvB-c1, ... so every LDWEIGHTS
  (~107ns) hides under the preceding 216ns same-weight matmul. In the v1
  per-pair order roughly half the projection matmuls paid an exposed
  LDWEIGHTS (+107ns each).
- psum->sbuf conversions are SINGLE 512-free ops (DVE/ACT fixed per-op
  cost ~340ns dominates; splitting by halves doubled op count for
  marginal latency gain).
- Everything else follows v1: x^T pair-major host layout (one
  128-descriptor DMA per pair), packed weight blob, scores as 3 causal
  blocks, exp-then-mask on GpSimd, V' = [V|1|1] ones-column denominator
  trick (host divides), per-pair out store, software-pipeline skew (now
  one GROUP of skew).

Negative results from this session (measured on HW, see transcript):
- Col-tiled K-projection (pairs at array cols 0:63/64:127) and row-tiled
  scores: tile-position alternation forces isolated-MM latency
  ((398+N)/2.4 instead of N/2.4 streaming) on every matmul around it,
  costing more than the array-occupancy win. Zero net overlap observed.
- Transposed out-product (M=66): M%32 != 0 breaks back-to-back MM
  streaming (275ns vs 56ns per out matmul).
- Mixing transpose-mode input base partitions (64 and 0) in one kernel
  crashes the NEFF at runtime; row-tiled matmuls writing the same psum
  bank also crash.
- GpSimd sbuf->sbuf copies are ~3x slower than DVE for [64, 512].
"""

import ml_dtypes
import numpy as np

import concourse.bacc as bacc
import concourse.mybir as mybir
import concourse.tile as tile
from concourse import bass_utils
from concourse.masks import make_identity

N_CORES = 8
B_FULL, T, C, H = 256, 256, 384, 64
B_SHARD = B_FULL // N_CORES  # 32
NPAIR = B_SHARD // 2  # 16
NGROUP = NPAIR // 2  # 8
F32 = mybir.dt.float32
BF16 = mybir.dt.bfloat16
SCALE = float(H) ** -0.5

EXP = mybir.ActivationFunctionType.Exp


def _build(has_bias):
    MMT = BF16
    nc = bacc.Bacc("TRN2", target_bir_lowering=False, debug=False, num_devices=N_CORES)

    # x^T pair-major: row (bp*128+p), cols (k*512+t) -> x[pair bp][t][k*128+p]
    xT_d = nc.dram_tensor("xT", [NPAIR * 128, 3 * 512], MMT, kind="ExternalInput").ap()
    # all matmul weights in one blob: cols [c*128 .. c*128+64) = Wq chunk c,
    # [c*128+64 ..) = Wv chunk c, cols [384+c*64) = Wk chunk c
    wp_d = nc.dram_tensor("wpack", [128, 576], MMT, kind="ExternalInput").ap()
    # biases: col 0 = [bq;bv], col 1 = [bk;bk]
    bias_d = (
        nc.dram_tensor("bias", [128, 2], F32, kind="ExternalInput").ap()
        if has_bias
        else None
    )
    # out pair-major: row (bp*128+p), cols ((bi*2+n)*66+h) -> item 2bp+bi,
    # t = n*128+p; col 64 of each 66-chunk is the softmax denominator and the
    # host does the divide (keeps reciprocal+multiply off the device)
    out_d = nc.dram_tensor("out", [NPAIR * 128, 264], MMT, kind="ExternalOutput").ap()

    xT_r = xT_d.rearrange("(b p) (k t) -> b p k t", p=128, k=3)
    out_r = out_d.rearrange("(b p) (i n h) -> b p i n h", p=128, i=2, n=2)

    with tile.TileContext(nc) as tc:
        with (
            tc.tile_pool(name="singles", bufs=1) as singles,
            tc.tile_pool(name="sb", bufs=2) as sb,
            tc.tile_pool(name="sbx", bufs=2) as sbx,
            tc.tile_pool(name="ps_qv", bufs=1, space="PSUM") as ps_qv,
            tc.tile_pool(name="ps_k", bufs=1, space="PSUM") as ps_k,
            tc.tile_pool(name="ps_s", bufs=2, space="PSUM") as ps_s,
            tc.tile_pool(name="ps_v", bufs=1, space="PSUM") as ps_v,
            tc.tile_pool(name="ps_o", bufs=1, space="PSUM") as ps_o,
        ):
            # ---- one-time setup ----
            # pair 0 chunk 0 first, then the first weight chunk: these gate
            # the first matmul, so their descriptors hit the queues first
            xtA0 = sbx.tile([128, 3, 512], MMT, tag="xtA")
            nc.sync.dma_start(xtA0[:, 0, :], xT_r[0, :, 0, :])
            wt = singles.tile([128, 576], MMT)
            nc.sync.dma_start(wt[:, 0:128], wp_d[:, 0:128])
            nc.sync.dma_start(wt[:, 128:576], wp_d[:, 128:576])
            wqv = wt[:, 0:384].rearrange("p (c m) -> p c m", c=3)  # [128, 3, 128]
            wkk = wt[:, 384:576].rearrange("p (c m) -> p c m", c=3)  # [128, 3, 64]
            if has_bias:
                bias_t = singles.tile([128, 2], F32)
                nc.sync.dma_start(bias_t[:], bias_d[:])

            identf = singles.tile([128, 128], F32)
            make_identity(nc, identf[:])
            ident = singles.tile([128, 128], MMT)
            nc.vector.tensor_copy(ident[:], identf[:])

            def stage_a(g):
                """x DMAs + chunk-major projections + conversions, group g
                (pairs 2g, 2g+1)."""
                if g == 0:
                    xtA = xtA0
                    nc.sync.dma_start(xtA[:, 1:3, :], xT_r[0, :, 1:3, :])
                else:
                    xtA = sbx.tile([128, 3, 512], MMT, tag="xtA")
                    nc.sync.dma_start(xtA[:], xT_r[2 * g])
                xtB = sbx.tile([128, 3, 512], MMT, tag="xtB")
                nc.sync.dma_start(xtB[:], xT_r[2 * g + 1])

                qv_ps = ps_qv.tile([128, 2, 512], F32, tag="qv_ps")
                kA_ps = ps_k.tile([64, 512], F32, tag="kA")
                kB_ps = ps_k.tile([64, 512], F32, tag="kB")
                # chunk-major: consecutive matmuls share the stationary
                # weights, so each auto-emitted LDWEIGHTS hides under the
                # previous same-weight 216ns matmul stream.
                for c in range(3):
                    nc.tensor.matmul(
                        qv_ps[:, 0, :], wqv[:, c, :], xtA[:, c, :],
                        start=(c == 0), stop=(c == 2),
                    )
                    nc.tensor.matmul(
                        qv_ps[:, 1, :], wqv[:, c, :], xtB[:, c, :],
                        start=(c == 0), stop=(c == 2),
                    )
                for c in range(3):
                    nc.tensor.matmul(
                        kA_ps[:], wkk[:, c, :], xtA[:, c, :],
                        start=(c == 0), stop=(c == 2),
                    )
                    nc.tensor.matmul(
                        kB_ps[:], wkk[:, c, :], xtB[:, c, :],
                        start=(c == 0), stop=(c == 2),
                    )

                qv2A = sb.tile([128, 512], MMT, tag="qv2A")
                qv2B = sb.tile([128, 512], MMT, tag="qv2B")
                k2A = sb.tile([64, 512], MMT, tag="k2A")
                k2B = sb.tile([64, 512], MMT, tag="k2B")
                if has_bias:
                    nc.vector.tensor_single_scalar(
                        qv2A[:], qv_ps[:, 0, :], bias_t[:, 0:1], op=mybir.AluOpType.add
                    )
                    nc.vector.tensor_single_scalar(
                        qv2B[:], qv_ps[:, 1, :], bias_t[:, 0:1], op=mybir.AluOpType.add
                    )
                    nc.scalar.tensor_single_scalar(
                        k2A[:], kA_ps[:], bias_t[0:64, 1:2], op=mybir.AluOpType.add
                    )
                    nc.scalar.tensor_single_scalar(
                        k2B[:], kB_ps[:], bias_t[0:64, 1:2], op=mybir.AluOpType.add
                    )
                else:
                    nc.vector.tensor_copy(qv2A[:], qv_ps[:, 0, :])
                    nc.vector.tensor_copy(qv2B[:], qv_ps[:, 1, :])
                    nc.scalar.copy(k2A[:], kA_ps[:])
                    nc.scalar.copy(k2B[:], kB_ps[:])
                return (qv2A, k2A), (qv2B, k2B)

            def stage_b(pa, qv2, k2):
                """scores/softmax/V for one pair (v1 baseline structure)."""
                v_ps = ps_v.tile([128, 2, 4, 64], MMT, tag="v_ps", bufs=1)
                W = sb.tile([128, 2, 3, 128], MMT, tag=f"W{pa}")

                def transp(q):
                    nc.tensor.transpose(
                        v_ps[:, pa, q, :],
                        qv2[64:128, q * 128 : (q + 1) * 128],
                        ident[64:128, 64:128],
                    )

                s_tiles = []

                def scores(bi):
                    toff = bi * 256
                    qT = qv2[0:64, toff : toff + 256]
                    kT = k2[0:64, toff : toff + 256]
                    s_ps = ps_s.tile([128, 3, 128], F32, tag="s_ps")
                    s_tiles.append(s_ps)
                    nc.tensor.matmul(
                        s_ps[:, 0:2, :], kT[:, 0:128], qT, start=True, stop=True
                    )
                    nc.tensor.matmul(
                        s_ps[:, 2, :], kT[:, 128:256], qT[:, 128:256],
                        start=True, stop=True,
                    )

                def softmax(bi):
                    nc.scalar.activation(
                        W[:, bi, :, :], s_tiles[bi][:], EXP, scale=SCALE
                    )
                    nc.gpsimd.affine_select(
                        out=W[:, bi, 0:3:2, :],
                        in_=W[:, bi, 0:3:2, :],
                        compare_op=mybir.AluOpType.is_ge,
                        fill=0.0,
                        base=0,
                        pattern=[[0, 2], [1, 128]],  # keep where (-s + t) >= 0
                        channel_multiplier=-1,
                    )

                transp(0)
                transp(1)
                scores(0)
                transp(2)
                transp(3)
                softmax(0)
                scores(1)
                softmax(1)

                v_sb = sb.tile([128, 2, 2, 66], MMT, tag=f"v_sb{pa}", bufs=2)
                nc.vector.tensor_copy(
                    v_sb[:, :, :, 0:64].rearrange("p i n h -> p (i n) h"),
                    v_ps[:, pa],
                )
                # ones cols -> out cols 64:66 = softmax denominator
                nc.vector.memset(v_sb[:, :, :, 64:66], 1.0)
                return W, v_sb

            def stage_c(bp, pa, W, v_sb):
                """out matmuls + store for pair bp (v1 baseline structure)."""
                o_ps = ps_o.tile([128, 2, 2, 66], F32, tag="o_ps")
                for bi in range(2):
                    nc.tensor.matmul(
                        o_ps[:, bi, 0, :], W[:, bi, 0, :], v_sb[:, bi, 0, :],
                        start=True, stop=True,
                    )
                    nc.tensor.matmul(
                        o_ps[:, bi, 1, :], W[:, bi, 1, :], v_sb[:, bi, 0, :],
                        start=True, stop=False,
                    )
                    nc.tensor.matmul(
                        o_ps[:, bi, 1, :], W[:, bi, 2, :], v_sb[:, bi, 1, :],
                        start=False, stop=True,
                    )

                o_sb = sb.tile([128, 2, 2, 66], MMT, tag=f"o_sb{pa}")
                cp = nc.vector.tensor_copy if pa == 0 else nc.scalar.copy
                if bp == NPAIR - 1:
                    # last pair: store per item so item0 drains while item1
                    # finishes (the tail often runs at HAM half clock)
                    for bi in range(2):
                        cp(o_sb[:, bi], o_ps[:, bi])
                        nc.sync.dma_start(out_r[bp, :, bi], o_sb[:, bi])
                else:
                    cp(o_sb[:], o_ps[:])
                    nc.sync.dma_start(out_r[bp], o_sb[:])

            # software pipeline with a 1-group skew: emitting group g+1's
            # projections before group g's attention keeps the PE queue fed
            # while stage B waits on cross-engine dependencies.
            pend = stage_a(0)
            for g in range(NGROUP):
                nxt = stage_a(g + 1) if g + 1 < NGROUP else None
                for pa in range(2):
                    wv = stage_b(pa, *pend[pa])
                    stage_c(2 * g + pa, pa, *wv)
                pend = nxt

    nc.compile()
    return nc


_CACHE = {}


def get_nc(has_bias=False):
    if has_bias not in _CACHE:
        _CACHE[has_bias] = _build(has_bias)
    return _CACHE[has_bias]


def make_in_maps(x, Wq, bq, Wk, bk, Wv, bv):
    bf16 = ml_dtypes.bfloat16
    x = np.asarray(x, dtype=np.float32)
    Wq = np.asarray(Wq, dtype=np.float32)
    Wk = np.asarray(Wk, dtype=np.float32)
    Wv = np.asarray(Wv, dtype=np.float32)
    # weight blob: per c-chunk [Wq | Wv] then the 3 Wk chunks
    wp = np.zeros((128, 576), dtype=np.float32)
    for c in range(3):
        wp[:, c * 128 : c * 128 + 64] = Wq[c * 128 : (c + 1) * 128]
        wp[:, c * 128 + 64 : c * 128 + 128] = Wv[c * 128 : (c + 1) * 128]
        wp[:, 384 + c * 64 : 384 + (c + 1) * 64] = Wk[c * 128 : (c + 1) * 128]
    wp = np.ascontiguousarray(wp.astype(bf16))
    bq = np.asarray(bq, dtype=np.float32).ravel()
    bk = np.asarray(bk, dtype=np.float32).ravel()
    bv = np.asarray(bv, dtype=np.float32).ravel()
    has_bias = bool(np.any(bq) or np.any(bk) or np.any(bv))
    bias = np.zeros((128, 2), dtype=np.float32)
    bias[0:64, 0] = bq
    bias[64:128, 0] = bv
    bias[0:64, 1] = bk
    bias[64:128, 1] = bk

    in_maps = []
    for i in range(N_CORES):
        shard = x[i * B_SHARD : (i + 1) * B_SHARD]  # [32, 256, 384]
        pairs = shard.reshape(NPAIR, 512, C)  # t within pair = bi*256 + t'
        # [b, p, k, t] with c = k*128 + p
        xTc = pairs.transpose(0, 2, 1)  # [b, C, t]
        xT = xTc.reshape(NPAIR, 3, 128, 512).transpose(0, 2, 1, 3)
        xT = np.ascontiguousarray(xT.astype(bf16)).reshape(NPAIR * 128, 3 * 512)
        m = {"xT": xT, "wpack": wp}
        if has_bias:
            m["bias"] = bias
        in_maps.append(m)
    return in_maps


def kernel(x, Wq, bq, Wk, bk, Wv, bv):
    in_maps = make_in_maps(x, Wq, bq, Wk, bk, Wv, bv)
    nc = get_nc("bias" in in_maps[0])
    res = bass_utils.run_bass_kernel_spmd(nc, in_maps, core_ids=list(range(N_CORES)))
    outs = []
    for i in range(N_CORES):
        r = res.results[i]["out"].reshape(NPAIR, 128, 2, 2, 66).astype(np.float32)
        r = r[..., 0:64] / r[..., 64:65]  # softmax denominator divide
        # [b, p, i, n, h] -> item 2b+i, t = n*128+p
        outs.append(
            np.ascontiguousarray(r.transpose(0, 2, 3, 1, 4)).reshape(B_SHARD, T, H)
        )
    return np.concatenate(outs, axis=0)


# revision 11
# speedup vs baseline: 1.3507x; 1.0519x over previous
"""Causal single-head attention on 8 trn2 cores, batch-data-parallel.

Computes, for each batch item b:
    Q = x[b] @ Wq + bq; K = x[b] @ Wk + bk; V = x[b] @ Wv + bv
    out[b] = softmax(causal_mask(Q K^T / sqrt(H))) @ V

Shapes: x [256, 256, 384], W* [384, 64], b* [64], out [256, 256, 64] fp32.
Sharding: batch axis split across 8 cores (32 items each), weights replicated.

All matmul operands are bfloat16 (PSUM accumulation stays fp32); rel err
~5e-3 against the fp32 reference (gate 2e-2).

v4 structure = v1 baseline + chunk-major projections over 2-pair groups:
- Pairs are processed in GROUPS of two (A, B). The projection matmuls run
  CHUNK-MAJOR: qvA-c0, qvB-c0, qvA-c1, qvB-c1, ... so every LDWEIGHTS
  (~107ns) hides under the preceding 216ns same-weight matmul. In the v1
  per-pair order roughly half the projection matmuls paid an exposed
  LDWEIGHTS (+107ns each).
- psum->sbuf conversions are SINGLE 512-free ops (DVE/ACT fixed per-op
  cost ~340ns dominates; splitting by halves doubled op count for
  marginal latency gain).
- Everything else follows v1: x^T pair-major host layout (one
  128-descriptor DMA per pair), packed weight blob, scores as 3 causal
  blocks, exp-then-mask on GpSimd, V' = [V|1|1] ones-column denominator
  trick (host divides), per-pair out store, software-pipeline skew (now
  one GROUP of skew).

Negative results from this session (measured on HW, see transcript):
- Col-tiled K-projection (pairs at array cols 0:63/64:127) and row-tiled
  scores: tile-position alternation forces isolated-MM latency
  ((398+N)/2.4 instead of N/2.4 streaming) on every matmul around it,
  costing more than the array-occupancy win. Zero net overlap observed.
- Transposed out-product (M=66): M%32 != 0 breaks back-to-back MM
  streaming (275ns vs 56ns per out matmul).
- Mixing transpose-mode input base partitions (64 and 0) in one kernel
  crashes the NEFF at runtime; row-tiled matmuls writing the same psum
  bank also crash.
- GpSimd sbuf->sbuf copies are ~3x slower than DVE for [64, 512].
"""

import ml_dtypes
import numpy as np

import concourse.bacc as bacc
import concourse.mybir as mybir
import concourse.tile as tile
from concourse import bass_utils
from concourse.masks import make_identity

N_CORES = 8
B_FULL, T, C, H = 256, 256, 384, 64
B_SHARD = B_FULL // N_CORES  # 32
NPAIR = B_SHARD // 2  # 16
NGROUP = NPAIR // 2  # 8
F32 = mybir.dt.float32
BF16 = mybir.dt.bfloat16
SCALE = float(H) ** -0.5

EXP = mybir.ActivationFunctionType.Exp


def _build(has_bias):
    MMT = BF16
    nc = bacc.Bacc("TRN2", target_bir_lowering=False, debug=False, num_devices=N_CORES)

    # x^T pair-major: row (bp*128+p), cols (k*512+t) -> x[pair bp][t][k*128+p]
    xT_d = nc.dram_tensor("xT", [NPAIR * 128, 3 * 512], MMT, kind="ExternalInput").ap()
    # all matmul weights in one blob: cols [c*128 .. c*128+64) = Wq chunk c,
    # [c*128+64 ..) = Wv chunk c, cols [384+c*64) = Wk chunk c
    wp_d = nc.dram_tensor("wpack", [128, 576], MMT, kind="ExternalInput").ap()
    # biases: col 0 = [bq;bv], col 1 = [bk;bk]
    bias_d = (
        nc.dram_tensor("bias", [128, 2], F32, kind="ExternalInput").ap()
        if has_bias
        else None
    )
    # out pair-major: row (bp*128+p), cols ((bi*2+n)*66+h) -> item 2bp+bi,
    # t = n*128+p; col 64 of each 66-chunk is the softmax denominator and the
    # host does the divide (keeps reciprocal+multiply off the device)
    out_d = nc.dram_tensor("out", [NPAIR * 128, 264], MMT, kind="ExternalOutput").ap()

    xT_r = xT_d.rearrange("(b p) (k t) -> b p k t", p=128, k=3)
    out_r = out_d.rearrange("(b p) (i n h) -> b p i n h", p=128, i=2, n=2)

    with tile.TileContext(nc) as tc:
        with (
            tc.tile_pool(name="singles", bufs=1) as singles,
            tc.tile_pool(name="sb", bufs=2) as sb,
            tc.tile_pool(name="sbx", bufs=2) as sbx,
            tc.tile_pool(name="ps_qv", bufs=1, space="PSUM") as ps_qv,
            tc.tile_pool(name="ps_k", bufs=1, space="PSUM") as ps_k,
            tc.tile_pool(name="ps_s", bufs=2, space="PSUM") as ps_s,
            tc.tile_pool(name="ps_v", bufs=1, space="PSUM") as ps_v,
            tc.tile_pool(name="ps_o", bufs=1, space="PSUM") as ps_o,
        ):
            # ---- one-time setup ----
            # pair 0 chunk 0 first, then the first weight chunk: these gate
            # the first matmul, so their descriptors hit the queues first
            xtA0 = sbx.tile([128, 3, 512], MMT, tag="xtA")
            nc.sync.dma_start(xtA0[:, 0, :], xT_r[0, :, 0, :])
            wt = singles.tile([128, 576], MMT)
            nc.sync.dma_start(wt[:, 0:128], wp_d[:, 0:128])
            nc.sync.dma_start(wt[:, 128:576], wp_d[:, 128:576])
            wqv = wt[:, 0:384].rearrange("p (c m) -> p c m", c=3)  # [128, 3, 128]
            wkk = wt[:, 384:576].rearrange("p (c m) -> p c m", c=3)  # [128, 3, 64]
            if has_bias:
                bias_t = singles.tile([128, 2], F32)
                nc.sync.dma_start(bias_t[:], bias_d[:])

            identf = singles.tile([128, 128], F32)
            make_identity(nc, identf[:])
            ident = singles.tile([128, 128], MMT)
            nc.vector.tensor_copy(ident[:], identf[:])

            def stage_a_mm(g):
                """x DMAs + chunk-major projections, group g (pairs 2g, 2g+1)."""
                if g == 0:
                    xtA = xtA0
                    nc.sync.dma_start(xtA[:, 1:3, :], xT_r[0, :, 1:3, :])
                else:
                    xtA = sbx.tile([128, 3, 512], MMT, tag="xtA")
                    nc.sync.dma_start(xtA[:], xT_r[2 * g])
                xtB = sbx.tile([128, 3, 512], MMT, tag="xtB")
                nc.sync.dma_start(xtB[:], xT_r[2 * g + 1])

                qv_ps = ps_qv.tile([128, 2, 512], F32, tag="qv_ps")
                kA_ps = ps_k.tile([64, 512], F32, tag="kA")
                kB_ps = ps_k.tile([64, 512], F32, tag="kB")
                # chunk-major: consecutive matmuls share the stationary
                # weights, so each auto-emitted LDWEIGHTS hides under the
                # previous same-weight 216ns matmul stream.
                for c in range(3):
                    nc.tensor.matmul(
                        qv_ps[:, 0, :], wqv[:, c, :], xtA[:, c, :],
                        start=(c == 0), stop=(c == 2),
                    )
                    nc.tensor.matmul(
                        qv_ps[:, 1, :], wqv[:, c, :], xtB[:, c, :],
                        start=(c == 0), stop=(c == 2),
                    )
                for c in range(3):
                    nc.tensor.matmul(
                        kA_ps[:], wkk[:, c, :], xtA[:, c, :],
                        start=(c == 0), stop=(c == 2),
                    )
                    nc.tensor.matmul(
                        kB_ps[:], wkk[:, c, :], xtB[:, c, :],
                        start=(c == 0), stop=(c == 2),
                    )

                return qv_ps, kA_ps, kB_ps

            def stage_a_conv(g, ps):
                """psum->sbuf conversions for group g — emitted AFTER group
                g-1's EXPs so the ACT queue doesn't head-of-line block the
                softmax on next-group projections."""
                qv_ps, kA_ps, kB_ps = ps
                qv2A = sb.tile([128, 512], MMT, tag="qv2A")
                qv2B = sb.tile([128, 512], MMT, tag="qv2B")
                k2A = sb.tile([64, 512], MMT, tag="k2A")
                k2B = sb.tile([64, 512], MMT, tag="k2B")
                if has_bias:
                    nc.vector.tensor_single_scalar(
                        qv2A[:], qv_ps[:, 0, :], bias_t[:, 0:1], op=mybir.AluOpType.add
                    )
                    nc.vector.tensor_single_scalar(
                        qv2B[:], qv_ps[:, 1, :], bias_t[:, 0:1], op=mybir.AluOpType.add
                    )
                    nc.scalar.tensor_single_scalar(
                        k2A[:], kA_ps[:], bias_t[0:64, 1:2], op=mybir.AluOpType.add
                    )
                    nc.scalar.tensor_single_scalar(
                        k2B[:], kB_ps[:], bias_t[0:64, 1:2], op=mybir.AluOpType.add
                    )
                else:
                    nc.vector.tensor_copy(qv2A[:], qv_ps[:, 0, :])
                    nc.vector.tensor_copy(qv2B[:], qv_ps[:, 1, :])
                    nc.scalar.copy(k2A[:], kA_ps[:])
                    nc.scalar.copy(k2B[:], kB_ps[:])
                return (qv2A, k2A), (qv2B, k2B)

            def stage_b(g, convs):
                """scores/softmax/V for BOTH pairs of group g, emitted as
                same-class blocks (transposes, then scores) to minimise PE
                config switches; exp/mask run on ACT/GpSimd underneath."""
                (qv2A, k2A), (qv2B, k2B) = convs
                qv2 = (qv2A, qv2B)
                k2 = (k2A, k2B)
                v_ps = ps_v.tile([128, 2, 4, 64], MMT, tag="v_ps", bufs=1)
                W = [
                    sb.tile([128, 2, 3, 128], MMT, tag="WA", name="W_A"),
                    sb.tile([128, 2, 3, 128], MMT, tag="WB", name="W_B"),
                ]
                s_tiles = {}

                def scores(pa, bi):
                    toff = bi * 256
                    qT = qv2[pa][0:64, toff : toff + 256]
                    kT = k2[pa][0:64, toff : toff + 256]
                    s_ps = ps_s.tile([128, 3, 128], F32, tag="s_ps")
                    s_tiles[(pa, bi)] = s_ps
                    nc.tensor.matmul(
                        s_ps[:, 0:2, :], kT[:, 0:128], qT, start=True, stop=True
                    )
                    nc.tensor.matmul(
                        s_ps[:, 2, :], kT[:, 128:256], qT[:, 128:256],
                        start=True, stop=True,
                    )

                def softmax(pa, bi):
                    nc.scalar.activation(
                        W[pa][:, bi, :, :], s_tiles[(pa, bi)][:], EXP, scale=SCALE
                    )
                    nc.gpsimd.affine_select(
                        out=W[pa][:, bi, 0:3:2, :],
                        in_=W[pa][:, bi, 0:3:2, :],
                        compare_op=mybir.AluOpType.is_ge,
                        fill=0.0,
                        base=0,
                        pattern=[[0, 2], [1, 128]],  # keep where (-s + t) >= 0
                        channel_multiplier=-1,
                    )

                # one uninterrupted transpose-mode block
                for pa in range(2):
                    for q in range(4):
                        nc.tensor.transpose(
                            v_ps[:, pa, q, :],
                            qv2[pa][64:128, q * 128 : (q + 1) * 128],
                            ident[64:128, 64:128],
                        )
                # one scores block; EXPs/masks chase it on ACT/GpSimd
                scores(0, 0)
                scores(1, 0)
                softmax(0, 0)
                softmax(1, 0)
                scores(0, 1)
                scores(1, 1)
                softmax(0, 1)
                softmax(1, 1)

                v_sbs = []
                for pa in range(2):
                    v_sb = sb.tile(
                        [128, 2, 2, 66], MMT, tag=f"v_sb{pa}", bufs=2, name="v_sb"
                    )
                    nc.vector.tensor_copy(
                        v_sb[:, :, :, 0:64].rearrange("p i n h -> p (i n) h"),
                        v_ps[:, pa],
                    )
                    # ones cols -> out cols 64:66 = softmax denominator
                    nc.gpsimd.memset(v_sb[:, :, :, 64:66], 1.0)
                    v_sbs.append(v_sb)
                return W, v_sbs

            def stage_c(g, W, v_sbs):
                """out matmuls + store for both pairs of group g — runs with
                a ONE-GROUP skew after stage_b so the exp/mask chain has a
                full projections+scores span to complete."""
                for pa in range(2):
                    bp = 2 * g + pa
                    v_sb = v_sbs[pa]
                    o_ps = ps_o.tile([128, 2, 2, 66], F32, tag="o_ps")
                    for bi in range(2):
                        nc.tensor.matmul(
                            o_ps[:, bi, 0, :], W[pa][:, bi, 0, :], v_sb[:, bi, 0, :],
                            start=True, stop=True,
                        )
                        nc.tensor.matmul(
                            o_ps[:, bi, 1, :], W[pa][:, bi, 1, :], v_sb[:, bi, 0, :],
                            start=True, stop=False,
                        )
                        nc.tensor.matmul(
                            o_ps[:, bi, 1, :], W[pa][:, bi, 2, :], v_sb[:, bi, 1, :],
                            start=False, stop=True,
                        )

                    o_sb = sb.tile([128, 2, 2, 66], MMT, tag=f"o_sb{pa}", name="o_sb")
                    cp = nc.vector.tensor_copy if pa == 0 else nc.scalar.copy
                    if bp == NPAIR - 1:
                        # last pair: store per item so item0 drains while
                        # item1 finishes (the tail often runs at half clock)
                        for bi in range(2):
                            cp(o_sb[:, bi], o_ps[:, bi])
                            nc.sync.dma_start(out_r[bp, :, bi], o_sb[:, bi])
                    else:
                        cp(o_sb[:], o_ps[:])
                        nc.sync.dma_start(out_r[bp], o_sb[:])

            # software pipeline, PE queue order per iteration:
            #   [proj g+1][transp+scores g][out g-1]
            # conversions for g+1 are emitted after g's EXPs (ACT is FIFO:
            # earlier emission would head-of-line block the softmax), and
            # the out stage runs one group behind scores so its exp/mask
            # inputs are ready when the PE reaches it.
            ps0 = stage_a_mm(0)
            conv = stage_a_conv(0, ps0)
            pend_c = None
            for g in range(NGROUP):
                ps_n = stage_a_mm(g + 1) if g + 1 < NGROUP else None
                wv = stage_b(g, conv)
                if ps_n is not None:
                    conv = stage_a_conv(g + 1, ps_n)
                if pend_c is not None:
                    stage_c(g - 1, *pend_c)
                pend_c = wv
            stage_c(NGROUP - 1, *pend_c)

    nc.compile()
    return nc


_CACHE = {}


def get_nc(has_bias=False):
    if has_bias not in _CACHE:
        _CACHE[has_bias] = _build(has_bias)
    return _CACHE[has_bias]


def make_in_maps(x, Wq, bq, Wk, bk, Wv, bv):
    bf16 = ml_dtypes.bfloat16
    x = np.asarray(x, dtype=np.float32)
    Wq = np.asarray(Wq, dtype=np.float32)
    Wk = np.asarray(Wk, dtype=np.float32)
    Wv = np.asarray(Wv, dtype=np.float32)
    # weight blob: per c-chunk [Wq | Wv] then the 3 Wk chunks
    wp = np.zeros((128, 576), dtype=np.float32)
    for c in range(3):
        wp[:, c * 128 : c * 128 + 64] = Wq[c * 128 : (c + 1) * 128]
        wp[:, c * 128 + 64 : c * 128 + 128] = Wv[c * 128 : (c + 1) * 128]
        wp[:, 384 + c * 64 : 384 + (c + 1) * 64] = Wk[c * 128 : (c + 1) * 128]
    wp = np.ascontiguousarray(wp.astype(bf16))
    bq = np.asarray(bq, dtype=np.float32).ravel()
    bk = np.asarray(bk, dtype=np.float32).ravel()
    bv = np.asarray(bv, dtype=np.float32).ravel()
    has_bias = bool(np.any(bq) or np.any(bk) or np.any(bv))
    bias = np.zeros((128, 2), dtype=np.float32)
    bias[0:64, 0] = bq
    bias[64:128, 0] = bv
    bias[0:64, 1] = bk
    bias[64:128, 1] = bk

    in_maps = []
    for i in range(N_CORES):
        shard = x[i * B_SHARD : (i + 1) * B_SHARD]  # [32, 256, 384]
        pairs = shard.reshape(NPAIR, 512, C)  # t within pair = bi*256 + t'
        # [b, p, k, t] with c = k*128 + p
        xTc = pairs.transpose(0, 2, 1)  # [b, C, t]
        xT = xTc.reshape(NPAIR, 3, 128, 512).transpose(0, 2, 1, 3)
        xT = np.ascontiguousarray(xT.astype(bf16)).reshape(NPAIR * 128, 3 * 512)
        m = {"xT": xT, "wpack": wp}
        if has_bias:
            m["bias"] = bias
        in_maps.append(m)
    return in_maps


def kernel(x, Wq, bq, Wk, bk, Wv, bv):
    in_maps = make_in_maps(x, Wq, bq, Wk, bk, Wv, bv)
    nc = get_nc("bias" in in_maps[0])
    res = bass_utils.run_bass_kernel_spmd(nc, in_maps, core_ids=list(range(N_CORES)))
    outs = []
    for i in range(N_CORES):
        r = res.results[i]["out"].reshape(NPAIR, 128, 2, 2, 66).astype(np.float32)
        r = r[..., 0:64] / r[..., 64:65]  # softmax denominator divide
        # [b, p, i, n, h] -> item 2b+i, t = n*128+p
        outs.append(
            np.ascontiguousarray(r.transpose(0, 2, 3, 1, 4)).reshape(B_SHARD, T, H)
        )
    return np.concatenate(outs, axis=0)


# revision 12
# speedup vs baseline: 1.3817x; 1.0229x over previous
"""Causal single-head attention on 8 trn2 cores, batch-data-parallel.

Computes, for each batch item b:
    Q = x[b] @ Wq + bq; K = x[b] @ Wk + bk; V = x[b] @ Wv + bv
    out[b] = softmax(causal_mask(Q K^T / sqrt(H))) @ V

Shapes: x [256, 256, 384], W* [384, 64], b* [64], out [256, 256, 64] fp32.
Sharding: batch axis split across 8 cores (32 items each), weights replicated.

All matmul operands are bfloat16 (PSUM accumulation stays fp32); rel err
~5e-3 against the fp32 reference (gate 2e-2).

v4 structure = v1 baseline + chunk-major projections over 2-pair groups:
- Pairs are processed in GROUPS of two (A, B). The projection matmuls run
  CHUNK-MAJOR: qvA-c0, qvB-c0, qvA-c1, qvB-c1, ... so every LDWEIGHTS
  (~107ns) hides under the preceding 216ns same-weight matmul. In the v1
  per-pair order roughly half the projection matmuls paid an exposed
  LDWEIGHTS (+107ns each).
- psum->sbuf conversions are SINGLE 512-free ops (DVE/ACT fixed per-op
  cost ~340ns dominates; splitting by halves doubled op count for
  marginal latency gain).
- Everything else follows v1: x^T pair-major host layout (one
  128-descriptor DMA per pair), packed weight blob, scores as 3 causal
  blocks, exp-then-mask on GpSimd, V' = [V|1|1] ones-column denominator
  trick (host divides), per-pair out store, software-pipeline skew (now
  one GROUP of skew).

Negative results from this session (measured on HW, see transcript):
- Col-tiled K-projection (pairs at array cols 0:63/64:127) and row-tiled
  scores: tile-position alternation forces isolated-MM latency
  ((398+N)/2.4 instead of N/2.4 streaming) on every matmul around it,
  costing more than the array-occupancy win. Zero net overlap observed.
- Transposed out-product (M=66): M%32 != 0 breaks back-to-back MM
  streaming (275ns vs 56ns per out matmul).
- Mixing transpose-mode input base partitions (64 and 0) in one kernel
  crashes the NEFF at runtime; row-tiled matmuls writing the same psum
  bank also crash.
- GpSimd sbuf->sbuf copies are ~3x slower than DVE for [64, 512].
"""

import ml_dtypes
import numpy as np

import concourse.bacc as bacc
import concourse.mybir as mybir
import concourse.tile as tile
from concourse import bass_utils
from concourse.masks import make_identity

N_CORES = 8
B_FULL, T, C, H = 256, 256, 384, 64
B_SHARD = B_FULL // N_CORES  # 32
NPAIR = B_SHARD // 2  # 16
NGROUP = NPAIR // 2  # 8
F32 = mybir.dt.float32
BF16 = mybir.dt.bfloat16
SCALE = float(H) ** -0.5

EXP = mybir.ActivationFunctionType.Exp


def _build(has_bias):
    MMT = BF16
    nc = bacc.Bacc("TRN2", target_bir_lowering=False, debug=False, num_devices=N_CORES)

    # x^T pair-major: row (bp*128+p), cols (k*512+t) -> x[pair bp][t][k*128+p]
    xT_d = nc.dram_tensor("xT", [NPAIR * 128, 3 * 512], MMT, kind="ExternalInput").ap()
    # all matmul weights in one blob: cols [c*128 .. c*128+64) = Wq chunk c,
    # [c*128+64 ..) = Wv chunk c, cols [384+c*64) = Wk chunk c
    wp_d = nc.dram_tensor("wpack", [128, 576], MMT, kind="ExternalInput").ap()
    # biases: col 0 = [bq;bv], col 1 = [bk;bk]
    bias_d = (
        nc.dram_tensor("bias", [128, 2], F32, kind="ExternalInput").ap()
        if has_bias
        else None
    )
    # out pair-major: row (bp*128+p), cols ((bi*2+n)*66+h) -> item 2bp+bi,
    # t = n*128+p; col 64 of each 66-chunk is the softmax denominator and the
    # host does the divide (keeps reciprocal+multiply off the device)
    out_d = nc.dram_tensor("out", [NPAIR * 128, 264], MMT, kind="ExternalOutput").ap()

    xT_r = xT_d.rearrange("(b p) (k t) -> b p k t", p=128, k=3)
    out_r = out_d.rearrange("(b p) (i n h) -> b p i n h", p=128, i=2, n=2)

    with tile.TileContext(nc) as tc:
        with (
            tc.tile_pool(name="singles", bufs=1) as singles,
            tc.tile_pool(name="sb", bufs=2) as sb,
            tc.tile_pool(name="sbx", bufs=2) as sbx,
            tc.tile_pool(name="ps_qv", bufs=1, space="PSUM") as ps_qv,
            tc.tile_pool(name="ps_k", bufs=1, space="PSUM") as ps_k,
            tc.tile_pool(name="ps_s", bufs=2, space="PSUM") as ps_s,
            tc.tile_pool(name="ps_v", bufs=1, space="PSUM") as ps_v,
            tc.tile_pool(name="ps_o", bufs=1, space="PSUM") as ps_o,
        ):
            # ---- one-time setup ----
            # pair 0 chunk 0 first, then the first weight chunk: these gate
            # the first matmul, so their descriptors hit the queues first
            xtA0 = sbx.tile([128, 3, 512], MMT, tag="xtA")
            nc.sync.dma_start(xtA0[:, 0, :], xT_r[0, :, 0, :])
            wt = singles.tile([128, 576], MMT)
            nc.sync.dma_start(wt[:, 0:128], wp_d[:, 0:128])
            nc.sync.dma_start(wt[:, 128:576], wp_d[:, 128:576])
            wqv = wt[:, 0:384].rearrange("p (c m) -> p c m", c=3)  # [128, 3, 128]
            wkk = wt[:, 384:576].rearrange("p (c m) -> p c m", c=3)  # [128, 3, 64]
            if has_bias:
                bias_t = singles.tile([128, 2], F32)
                nc.sync.dma_start(bias_t[:], bias_d[:])

            identf = singles.tile([128, 128], F32)
            make_identity(nc, identf[:])
            ident = singles.tile([128, 128], MMT)
            nc.vector.tensor_copy(ident[:], identf[:])

            # HAM warmup: ~3.2us of dummy matmuls on the identity while the
            # first x/weight DMAs are still in flight. The PE clock gate
            # releases (1.2 -> 2.4 GHz) after ~3.4us of sustained activity,
            # so the first real matmuls start warm instead of paying ~2.5us
            # of half-clock ramp.
            warm_ps = ps_s.tile([128, 3, 128], F32, tag="s_ps", name="warm_ps")
            for _ in range(30):
                nc.tensor.matmul(
                    warm_ps[:, 0, :], ident[:], ident[:], start=True, stop=True
                )

            def stage_a_mm(g):
                """x DMAs + chunk-major projections, group g (pairs 2g, 2g+1)."""
                if g == 0:
                    xtA = xtA0
                    nc.sync.dma_start(xtA[:, 1:3, :], xT_r[0, :, 1:3, :])
                else:
                    xtA = sbx.tile([128, 3, 512], MMT, tag="xtA")
                    nc.sync.dma_start(xtA[:], xT_r[2 * g])
                xtB = sbx.tile([128, 3, 512], MMT, tag="xtB")
                nc.sync.dma_start(xtB[:], xT_r[2 * g + 1])

                qv_ps = ps_qv.tile([128, 2, 512], F32, tag="qv_ps")
                kA_ps = ps_k.tile([64, 512], F32, tag="kA")
                kB_ps = ps_k.tile([64, 512], F32, tag="kB")
                # chunk-major: consecutive matmuls share the stationary
                # weights, so each auto-emitted LDWEIGHTS hides under the
                # previous same-weight 216ns matmul stream.
                for c in range(3):
                    nc.tensor.matmul(
                        qv_ps[:, 0, :], wqv[:, c, :], xtA[:, c, :],
                        start=(c == 0), stop=(c == 2),
                    )
                    nc.tensor.matmul(
                        qv_ps[:, 1, :], wqv[:, c, :], xtB[:, c, :],
                        start=(c == 0), stop=(c == 2),
                    )
                # pair-major: psum-bank alternation between consecutive
                # matmuls costs ~77ns each (measured), so keep each K
                # accumulation chain contiguous in its own bank.
                for c in range(3):
                    nc.tensor.matmul(
                        kA_ps[:], wkk[:, c, :], xtA[:, c, :],
                        start=(c == 0), stop=(c == 2),
                    )
                for c in range(3):
                    nc.tensor.matmul(
                        kB_ps[:], wkk[:, c, :], xtB[:, c, :],
                        start=(c == 0), stop=(c == 2),
                    )

                return qv_ps, kA_ps, kB_ps

            def stage_a_conv(g, ps):
                """psum->sbuf conversions for group g — emitted AFTER group
                g-1's EXPs so the ACT queue doesn't head-of-line block the
                softmax on next-group projections."""
                qv_ps, kA_ps, kB_ps = ps
                qv2A = sb.tile([128, 512], MMT, tag="qv2A")
                qv2B = sb.tile([128, 512], MMT, tag="qv2B")
                k2A = sb.tile([64, 512], MMT, tag="k2A")
                k2B = sb.tile([64, 512], MMT, tag="k2B")
                if has_bias:
                    nc.vector.tensor_single_scalar(
                        qv2A[:], qv_ps[:, 0, :], bias_t[:, 0:1], op=mybir.AluOpType.add
                    )
                    nc.vector.tensor_single_scalar(
                        qv2B[:], qv_ps[:, 1, :], bias_t[:, 0:1], op=mybir.AluOpType.add
                    )
                    nc.scalar.tensor_single_scalar(
                        k2A[:], kA_ps[:], bias_t[0:64, 1:2], op=mybir.AluOpType.add
                    )
                    nc.scalar.tensor_single_scalar(
                        k2B[:], kB_ps[:], bias_t[0:64, 1:2], op=mybir.AluOpType.add
                    )
                else:
                    nc.vector.tensor_copy(qv2A[:], qv_ps[:, 0, :])
                    nc.vector.tensor_copy(qv2B[:], qv_ps[:, 1, :])
                    nc.scalar.copy(k2A[:], kA_ps[:])
                    nc.scalar.copy(k2B[:], kB_ps[:])
                return (qv2A, k2A), (qv2B, k2B)

            def stage_b(g, convs):
                """scores/softmax/V for BOTH pairs of group g, emitted as
                same-class blocks (transposes, then scores) to minimise PE
                config switches; exp/mask run on ACT/GpSimd underneath."""
                (qv2A, k2A), (qv2B, k2B) = convs
                qv2 = (qv2A, qv2B)
                k2 = (k2A, k2B)
                v_ps = ps_v.tile([128, 2, 4, 64], MMT, tag="v_ps", bufs=1)
                W = [
                    sb.tile([128, 2, 3, 128], MMT, tag="WA", name="W_A"),
                    sb.tile([128, 2, 3, 128], MMT, tag="WB", name="W_B"),
                ]
                s_tiles = {}

                def scores(pa, bi):
                    toff = bi * 256
                    qT = qv2[pa][0:64, toff : toff + 256]
                    kT = k2[pa][0:64, toff : toff + 256]
                    s_ps = ps_s.tile([128, 3, 128], F32, tag="s_ps")
                    s_tiles[(pa, bi)] = s_ps
                    nc.tensor.matmul(
                        s_ps[:, 0:2, :], kT[:, 0:128], qT, start=True, stop=True
                    )
                    nc.tensor.matmul(
                        s_ps[:, 2, :], kT[:, 128:256], qT[:, 128:256],
                        start=True, stop=True,
                    )

                def softmax(pa, bi):
                    nc.scalar.activation(
                        W[pa][:, bi, :, :], s_tiles[(pa, bi)][:], EXP, scale=SCALE
                    )
                    nc.gpsimd.affine_select(
                        out=W[pa][:, bi, 0:3:2, :],
                        in_=W[pa][:, bi, 0:3:2, :],
                        compare_op=mybir.AluOpType.is_ge,
                        fill=0.0,
                        base=0,
                        pattern=[[0, 2], [1, 128]],  # keep where (-s + t) >= 0
                        channel_multiplier=-1,
                    )

                def transp(pa, q):
                    nc.tensor.transpose(
                        v_ps[:, pa, q, :],
                        qv2[pa][64:128, q * 128 : (q + 1) * 128],
                        ident[64:128, 64:128],
                    )

                # per-pair interleave (v1-measured best: transposes fill the
                # PE while the k2/qv2 conversions drain, exp chases scores)
                for pa in range(2):
                    transp(pa, 0)
                    transp(pa, 1)
                    scores(pa, 0)
                    transp(pa, 2)
                    transp(pa, 3)
                    softmax(pa, 0)
                    scores(pa, 1)
                    softmax(pa, 1)

                v_sbs = []
                for pa in range(2):
                    v_sb = sb.tile(
                        [128, 2, 2, 66], MMT, tag=f"v_sb{pa}", bufs=2, name="v_sb"
                    )
                    nc.vector.tensor_copy(
                        v_sb[:, :, :, 0:64].rearrange("p i n h -> p (i n) h"),
                        v_ps[:, pa],
                    )
                    # ones cols -> out cols 64:66 = softmax denominator
                    nc.gpsimd.memset(v_sb[:, :, :, 64:66], 1.0)
                    v_sbs.append(v_sb)
                return W, v_sbs

            def stage_c(g, W, v_sbs):
                """out matmuls + store for both pairs of group g — runs with
                a ONE-GROUP skew after stage_b so the exp/mask chain has a
                full projections+scores span to complete."""
                for pa in range(2):
                    bp = 2 * g + pa
                    v_sb = v_sbs[pa]
                    o_ps = ps_o.tile([128, 2, 2, 66], F32, tag="o_ps")
                    for bi in range(2):
                        nc.tensor.matmul(
                            o_ps[:, bi, 0, :], W[pa][:, bi, 0, :], v_sb[:, bi, 0, :],
                            start=True, stop=True,
                        )
                        nc.tensor.matmul(
                            o_ps[:, bi, 1, :], W[pa][:, bi, 1, :], v_sb[:, bi, 0, :],
                            start=True, stop=False,
                        )
                        nc.tensor.matmul(
                            o_ps[:, bi, 1, :], W[pa][:, bi, 2, :], v_sb[:, bi, 1, :],
                            start=False, stop=True,
                        )

                    o_sb = sb.tile([128, 2, 2, 66], MMT, tag=f"o_sb{pa}", name="o_sb")
                    cp = nc.vector.tensor_copy if pa == 0 else nc.scalar.copy
                    if bp == NPAIR - 1:
                        # last pair: store per item so item0 drains while
                        # item1 finishes (the tail often runs at half clock)
                        for bi in range(2):
                            cp(o_sb[:, bi], o_ps[:, bi])
                            nc.sync.dma_start(out_r[bp, :, bi], o_sb[:, bi])
                    else:
                        cp(o_sb[:], o_ps[:])
                        nc.sync.dma_start(out_r[bp], o_sb[:])

            # software pipeline, PE queue order per iteration:
            #   [proj g+1][transp+scores g][out g-1]
            # conversions for g+1 are emitted after g's EXPs (ACT is FIFO:
            # earlier emission would head-of-line block the softmax), and
            # the out stage runs one group behind scores so its exp/mask
            # inputs are ready when the PE reaches it.
            ps0 = stage_a_mm(0)
            conv = stage_a_conv(0, ps0)
            pend_c = None
            for g in range(NGROUP):
                ps_n = stage_a_mm(g + 1) if g + 1 < NGROUP else None
                wv = stage_b(g, conv)
                if ps_n is not None:
                    conv = stage_a_conv(g + 1, ps_n)
                if pend_c is not None:
                    stage_c(g - 1, *pend_c)
                pend_c = wv
            stage_c(NGROUP - 1, *pend_c)

    nc.compile()
    return nc


_CACHE = {}


def get_nc(has_bias=False):
    if has_bias not in _CACHE:
        _CACHE[has_bias] = _build(has_bias)
    return _CACHE[has_bias]


def make_in_maps(x, Wq, bq, Wk, bk, Wv, bv):
    bf16 = ml_dtypes.bfloat16
    x = np.asarray(x, dtype=np.float32)
    Wq = np.asarray(Wq, dtype=np.float32)
    Wk = np.asarray(Wk, dtype=np.float32)
    Wv = np.asarray(Wv, dtype=np.float32)
    # weight blob: per c-chunk [Wq | Wv] then the 3 Wk chunks
    wp = np.zeros((128, 576), dtype=np.float32)
    for c in range(3):
        wp[:, c * 128 : c * 128 + 64] = Wq[c * 128 : (c + 1) * 128]
        wp[:, c * 128 + 64 : c * 128 + 128] = Wv[c * 128 : (c + 1) * 128]
        wp[:, 384 + c * 64 : 384 + (c + 1) * 64] = Wk[c * 128 : (c + 1) * 128]
    wp = np.ascontiguousarray(wp.astype(bf16))
    bq = np.asarray(bq, dtype=np.float32).ravel()
    bk = np.asarray(bk, dtype=np.float32).ravel()
    bv = np.asarray(bv, dtype=np.float32).ravel()
    has_bias = bool(np.any(bq) or np.any(bk) or np.any(bv))
    bias = np.zeros((128, 2), dtype=np.float32)
    bias[0:64, 0] = bq
    bias[64:128, 0] = bv
    bias[0:64, 1] = bk
    bias[64:128, 1] = bk

    in_maps = []
    for i in range(N_CORES):
        shard = x[i * B_SHARD : (i + 1) * B_SHARD]  # [32, 256, 384]
        pairs = shard.reshape(NPAIR, 512, C)  # t within pair = bi*256 + t'
        # [b, p, k, t] with c = k*128 + p
        xTc = pairs.transpose(0, 2, 1)  # [b, C, t]
        xT = xTc.reshape(NPAIR, 3, 128, 512).transpose(0, 2, 1, 3)
        xT = np.ascontiguousarray(xT.astype(bf16)).reshape(NPAIR * 128, 3 * 512)
        m = {"xT": xT, "wpack": wp}
        if has_bias:
            m["bias"] = bias
        in_maps.append(m)
    return in_maps


def kernel(x, Wq, bq, Wk, bk, Wv, bv):
    in_maps = make_in_maps(x, Wq, bq, Wk, bk, Wv, bv)
    nc = get_nc("bias" in in_maps[0])
    res = bass_utils.run_bass_kernel_spmd(nc, in_maps, core_ids=list(range(N_CORES)))
    outs = []
    for i in range(N_CORES):
        r = res.results[i]["out"].reshape(NPAIR, 128, 2, 2, 66).astype(np.float32)
        r = r[..., 0:64] / r[..., 64:65]  # softmax denominator divide
        # [b, p, i, n, h] -> item 2b+i, t = n*128+p
        outs.append(
            np.ascontiguousarray(r.transpose(0, 2, 3, 1, 4)).reshape(B_SHARD, T, H)
        )
    return np.concatenate(outs, axis=0)
